# revision 13
# baseline (speedup 1.0000x reference)
"""Trainium2 Bass kernel for nn_CudaGRULM — cross-core layer pipeline.

The GRU scan is LDWEIGHTS-bound (48 128x128 weight tiles per step). The
data-parallel baseline gives each matmul only B/8 = 2 batch columns; here we
pipeline LAYERS across cores with the FULL batch B=16 per step: core k hosts
layers 2k and 2k+1 (k = 0..5); a chunk of S timesteps passes through both
layers within one round, then hops to core k+1 via one AllGather per round.
Core 0 folds the embedding into its input (masked one-hot matmul); core 5
computes the final LN + logits. Cores 6, 7 run the same SPMD program on zero
weights. Inside the scan, r-gates are computed first so the z-gate matmuls
fill the PE while the r-chain runs on DVE/ACT.

The scan is fully python-unrolled: matmuls whose operands carry
register-offset access patterns (For_i loop variable) cost ~195ns each on HW
vs ~40-60ns with static APs, so static unrolling is worth the program size.
Rounds are python-unrolled (collectives cannot live inside hardware loops);
each round's AllGather writes its own Shared DRAM buffer (single-writer
rule). Garbage cells at pipeline fill/drain stay finite by induction (all
buffers memset once; round 0 skips the AG read), and per-(core,round) state
masks zero the scan state at each layer's first real chunk.
"""

from contextlib import ExitStack

import numpy as np

import concourse.bass as bass
import concourse.bacc as bacc
import concourse.tile as tile
from concourse import mybir
from concourse.bass_utils import run_bass_kernel_spmd

FP = mybir.dt.float32
BF = mybir.dt.bfloat16
AF = mybir.ActivationFunctionType
ALU = mybir.AluOpType

# test/experiment knobs (kernel() itself always runs the default config):
# NO_COLLECTIVES replaces the AllGather with a local DRAM copy (wrong output,
# used for timing A/B and TimelineSim); UNROLL_SCAN statically unrolls the
# scan loop (needed by TimelineSim, which has no register interpreter).
NO_COLLECTIVES = False
UNROLL_SCAN = True
# timing diagnostic: keep only the scan's matmuls (drop the DVE/ACT chain);
# output is garbage — used to separate PE stream cost from chain stalls
SCAN_MM_ONLY = False


class Cfg:
    def __init__(self, V=256, D=512, DEPTH=12, DI=512, B=16, T=2048,
                 n_cores=8, S=64, U=8, EPS=1e-5):
        self.V, self.D, self.DEPTH, self.DI, self.B, self.T = V, D, DEPTH, DI, B, T
        self.n_cores = n_cores
        self.S = S                      # scan steps per chunk
        self.U = U                      # scan unroll inside For_i
        self.NC = T // S                # chunks
        self.CC = S * B                 # chunk cols (token-major: col = t*B+b)
        self.EPS = EPS
        self.KD = D // 128              # 4
        self.KV = V // 128              # 2
        self.MZR = 2 * DI // 128        # 8
        self.MH = DI // 128             # 4
        self.NH = self.CC // 512        # 512-col groups per chunk
        # core k processes chunk c through layers 2k,2k+1 at round c+k;
        # logits on core 5 in the same round -> last round = (NC-1)+5
        self.R = self.NC + 5
        assert D == DI and self.CC % 512 == 0 and S % U == 0


def build_kernel(ctx: ExitStack, tc: "tile.TileContext", outs, ins, obs, cfg: Cfg):
    nc = tc.nc
    c = cfg
    KD, KV, MZR, MH, B, CC, S, U, NH = (c.KD, c.KV, c.MZR, c.MH, c.B, c.CC,
                                        c.S, c.U, c.NH)
    MR = MZR // 2
    lg_out = outs["logits"]

    persist = ctx.enter_context(tc.tile_pool(name="persist", bufs=1))
    sb = ctx.enter_context(tc.tile_pool(name="sb", bufs=2))
    dram = ctx.enter_context(tc.tile_pool(name="dram", bufs=2, space="DRAM"))
    ps_sc = ctx.enter_context(tc.tile_pool(name="ps_sc", bufs=2, space="PSUM"))
    ps_pr = ctx.enter_context(tc.tile_pool(name="ps_pr", bufs=2, space="PSUM"))
    ps_bc = ctx.enter_context(tc.tile_pool(name="ps_bc", bufs=4, space="PSUM"))

    # ---- constants ----
    iota2 = persist.tile([128, KV], FP)
    nc.sync.dma_start(iota2[:], ins["iota2"][:])
    onec_f = persist.tile([1, 128], FP)
    nc.sync.dma_start(onec_f[:], ins["ones_col"][:])
    onec_b = persist.tile([1, 128], BF)
    nc.vector.tensor_copy(onec_b[:], onec_f[:])
    onek_f = persist.tile([128, 1], FP)
    nc.sync.dma_start(onek_f[:], ins["ones_k"][:])
    onek_b = persist.tile([128, 1], BF)
    nc.vector.tensor_copy(onek_b[:], onek_f[:])
    e_sb = persist.tile([128, KV, c.D], BF)
    nc.sync.dma_start(e_sb[:], ins["E_lhsT"][:])
    et_sb = persist.tile([128, KD, c.V], BF)
    nc.sync.dma_start(et_sb[:], ins["ET_rhs"][:])
    bv_sb = persist.tile([1, c.V], BF)
    nc.sync.dma_start(bv_sb[:], ins["bv_row"][:])
    eps_sb = persist.tile([1, 1], FP)
    nc.vector.memset(eps_sb[:], float(c.EPS))
    m_emb = persist.tile([128, 1], FP)
    nc.sync.dma_start(m_emb[:], ins["m_emb"][:])
    m_keep = persist.tile([128, 1], FP)
    nc.sync.dma_start(m_keep[:], ins["m_keep"][:])
    m_st = persist.tile([128, c.R], FP)
    nc.sync.dma_start(m_st[:], ins["m_st"][:])

    # ---- per-slot weights (resident the whole run) ----
    def load_w(pfx):
        shapes = dict(uzr=([128, KD, 2 * c.DI], BF), uh=([128, KD, c.DI], BF),
                      wzr=([128, KD, 2 * c.DI], BF), wh=([128, KD, c.DI], BF),
                      wo=([128, KD, c.D], BF), bzr=([128, MZR], FP),
                      bh=([128, MH], FP))
        srcs = dict(uzr="uzrT", uh="uhT", wzr="wzrT", wh="whT", wo="woT",
                    bzr="bzr", bh="bh")
        w = {}
        for kk, (shp_, dt_) in shapes.items():
            w[kk] = persist.tile(shp_, dt_, name=pfx + kk)
            nc.sync.dma_start(w[kk][:], ins[pfx + srcs[kk]][:])
        return w

    WA = load_w("LA_")
    WB = load_w("LB_")

    # ---- streaming buffers (cells are serial within a round -> shared) ----
    inA = persist.tile([128, KD, CC], BF)      # slot A input chunk
    Aout = persist.tile([128, KD, CC], BF)     # layer-2k output
    Bout = persist.tile([128, KD, CC], BF)     # layer-2k+1 output (payload)
    stA = persist.tile([128, KD, B], FP)       # carried scan states
    stB = persist.tile([128, KD, B], FP)
    hsT = persist.tile([128, KD, (S + 1) * B], BF)   # scan out ring (shared)
    xzr = persist.tile([128, MZR, CC], BF)           # input proj (shared)
    xh = persist.tile([128, MH, CC], BF)
    nc.vector.memset(inA[:], 0.0)
    nc.vector.memset(Aout[:], 0.0)
    nc.vector.memset(Bout[:], 0.0)
    nc.vector.memset(stA[:], 0.0)
    nc.vector.memset(stB[:], 0.0)

    # per-core row offset of the previous rank inside AG outputs
    if NO_COLLECTIVES:
        off = c.n_cores - 1
    else:
        off_reg = nc.sync.alloc_register("prev_off")
        nc.sync.reg_load(off_reg, ins["prev_off"][0:1, 0:1])
        off = nc.sync.snap(off_reg, donate=True, min_val=0, max_val=c.n_cores - 1)

    def layer_norm_half(tsrc, h, hn):
        """LN of cols [h*512,(h+1)*512) of tsrc ([128,KD,CC] bf16) into hn."""
        cs = slice(h * 512, (h + 1) * 512)
        mean_ps = ps_bc.tile([128, 512], FP, tag="bc")
        for k in range(KD):
            nc.tensor.matmul(mean_ps[0:1, :], onek_b[:], tsrc[:, k, cs],
                             start=(k == 0), stop=(k == KD - 1))
        sq = sb.tile([128, KD, 512], BF, tag="sq", bufs=2)
        for k in range(KD):
            nc.scalar.activation(sq[:, k, :], tsrc[:, k, cs], AF.Square)
        mean_r = sb.tile([1, 512], FP, tag="lnrow", bufs=8)
        nc.vector.tensor_scalar(mean_r[:], mean_ps[0:1, :], 1.0 / c.D, None,
                                ALU.mult)
        sq_ps = ps_bc.tile([128, 512], FP, tag="bc")
        for k in range(KD):
            nc.tensor.matmul(sq_ps[0:1, :], onek_b[:], sq[:, k, :],
                             start=(k == 0), stop=(k == KD - 1))
        msq_r = sb.tile([1, 512], FP, tag="lnrow", bufs=8)
        nc.vector.tensor_scalar(msq_r[:], sq_ps[0:1, :], 1.0 / c.D, None,
                                ALU.mult)
        var_r = sb.tile([1, 512], FP, tag="lnrow", bufs=8)
        nc.vector.tensor_tensor(var_r[:], mean_r[:], mean_r[:], ALU.mult)
        nc.vector.tensor_tensor(var_r[:], msq_r[:], var_r[:], ALU.subtract)
        std_r = sb.tile([1, 512], FP, tag="lnrow", bufs=8)
        nc.scalar.activation(std_r[:], var_r[:], AF.Sqrt, bias=eps_sb[:])
        rstd_r = sb.tile([1, 512], FP, tag="lnrow", bufs=8)
        nc.vector.reciprocal(rstd_r[:], std_r[:])
        mr_r = sb.tile([1, 512], FP, tag="lnrow", bufs=8)
        nc.vector.tensor_tensor(mr_r[:], mean_r[:], rstd_r[:], ALU.mult)
        rb_ps = ps_bc.tile([128, 512], FP, tag="bc")
        nc.tensor.matmul(rb_ps[:], onec_f[:], rstd_r[:], start=True, stop=True)
        mrb_ps = ps_bc.tile([128, 512], FP, tag="bc")
        nc.tensor.matmul(mrb_ps[:], onec_f[:], mr_r[:], start=True, stop=True)
        for k in range(KD):
            nc.vector.tensor_tensor(hn[:, k, :], tsrc[:, k, cs], rb_ps[:],
                                    ALU.mult)
            nc.vector.tensor_tensor(hn[:, k, :], hn[:, k, :], mrb_ps[:],
                                    ALU.subtract)

    def cell(tsrc, tdst, W, st, mask_col):
        # 1. masked carried state -> step-0 state (bf16 ring col 0)
        nc.vector.tensor_scalar(st[:], st[:], mask_col, None, ALU.mult)
        nc.vector.tensor_copy(hsT[:, :, 0:B], st[:])
        # 2. LN + input projections, per 512-col group
        for h in range(NH):
            cs = slice(h * 512, (h + 1) * 512)
            hn = sb.tile([128, KD, 512], BF, tag="hn", bufs=3)
            layer_norm_half(tsrc, h, hn)
            for m in range(MZR):
                px = ps_pr.tile([128, 512], FP, tag="pr")
                for k in range(KD):
                    nc.tensor.matmul(px[:], W["wzr"][:, k, m * 128:(m + 1) * 128],
                                     hn[:, k, :], start=(k == 0),
                                     stop=(k == KD - 1))
                nc.scalar.activation(xzr[:, m, cs], px[:], AF.Identity,
                                     bias=W["bzr"][:, m:m + 1])
            for m in range(MH):
                px = ps_pr.tile([128, 512], FP, tag="pr")
                for k in range(KD):
                    nc.tensor.matmul(px[:], W["wh"][:, k, m * 128:(m + 1) * 128],
                                     hn[:, k, :], start=(k == 0),
                                     stop=(k == KD - 1))
                nc.scalar.activation(xh[:, m, cs], px[:], AF.Identity,
                                     bias=W["bh"][:, m:m + 1])
        # 3. the scan: r gates first; z matmuls fill PE during the r chain
        def scan_trip(it):
            prev = None
            for u in range(U):
                cin = bass.ds((it + u) * B, B)
                cout = bass.ds((it + u + 1) * B, B)
                if prev is None:
                    prev = st
                ps = ps_sc.tile([128, 192], FP, tag="sc")
                zrr = ps[:, 0:64]
                zrz = ps[:, 64:128]
                hp = ps[:, 128:192]
                for m in range(MR, MZR):
                    for k in range(KD):
                        nc.tensor.matmul(zrr[:, (m - MR) * B:(m - MR + 1) * B],
                                         W["uzr"][:, k, m * 128:(m + 1) * 128],
                                         hsT[:, k, cin],
                                         start=(k == 0), stop=(k == KD - 1))
                if SCAN_MM_ONLY:
                    for m in range(MR):
                        for k in range(KD):
                            nc.tensor.matmul(zrz[:, m * B:(m + 1) * B],
                                             W["uzr"][:, k, m * 128:(m + 1) * 128],
                                             hsT[:, k, cin],
                                             start=(k == 0), stop=(k == KD - 1))
                    for m in range(MH):
                        for k in range(KD):
                            nc.tensor.matmul(hp[:, m * B:(m + 1) * B],
                                             W["uh"][:, k, m * 128:(m + 1) * 128],
                                             hsT[:, k, cin],
                                             start=(k == 0), stop=(k == KD - 1))
                    prev = st
                    continue
                zs_r = sb.tile([128, MH, B], FP, tag="szsr", bufs=3)
                nc.vector.tensor_tensor(zs_r[:], zrr, xzr[:, MR:MZR, cin],
                                        ALU.add)
                za_r = sb.tile([128, MH, B], FP, tag="szar", bufs=3)
                nc.scalar.activation(za_r[:], zs_r[:], AF.Sigmoid)
                rh = sb.tile([128, KD, B], BF, tag="srh", bufs=3)
                nc.vector.tensor_tensor(rh[:], za_r[:], prev[:], ALU.mult)
                for m in range(MR):
                    for k in range(KD):
                        nc.tensor.matmul(zrz[:, m * B:(m + 1) * B],
                                         W["uzr"][:, k, m * 128:(m + 1) * 128],
                                         hsT[:, k, cin],
                                         start=(k == 0), stop=(k == KD - 1))
                zs_z = sb.tile([128, MH, B], FP, tag="szsz", bufs=3)
                nc.vector.tensor_tensor(zs_z[:], zrz, xzr[:, 0:MR, cin],
                                        ALU.add)
                za_z = sb.tile([128, MH, B], FP, tag="szaz", bufs=3)
                nc.scalar.activation(za_z[:], zs_z[:], AF.Sigmoid)
                for m in range(MH):
                    for k in range(KD):
                        nc.tensor.matmul(hp[:, m * B:(m + 1) * B],
                                         W["uh"][:, k, m * 128:(m + 1) * 128],
                                         rh[:, k, :],
                                         start=(k == 0), stop=(k == KD - 1))
                hs_t = sb.tile([128, MH, B], FP, tag="shst", bufs=3)
                nc.vector.tensor_tensor(hs_t[:], hp, xh[:, :, cin], ALU.add)
                hc = sb.tile([128, MH, B], FP, tag="shc", bufs=3)
                nc.scalar.activation(hc[:], hs_t[:], AF.Tanh)
                d1 = sb.tile([128, KD, B], FP, tag="sd1", bufs=3)
                nc.vector.tensor_tensor(d1[:], hc[:], prev[:], ALU.subtract)
                d2 = sb.tile([128, KD, B], FP, tag="sd2", bufs=3)
                nc.vector.tensor_tensor(d2[:], za_z[:], d1[:], ALU.mult)
                nxt = sb.tile([128, KD, B], FP, tag="snx", bufs=3)
                nc.vector.tensor_tensor(nxt[:], prev[:], d2[:], ALU.add)
                nc.vector.tensor_copy(hsT[:, :, cout], nxt[:])
                prev = nxt
            # carry fp32 state across trips / rounds
            if not SCAN_MM_ONLY:
                nc.vector.tensor_copy(st[:], prev[:])

        if UNROLL_SCAN:
            for it0 in range(0, S, U):
                scan_trip(it0)
        else:
            with tc.For_i(0, S, U, hint_engines=(mybir.EngineType.PE,)) as it:
                scan_trip(it)
        # 4. output projection + residual
        for h in range(NH):
            cs = slice(h * 512, (h + 1) * 512)
            for dm in range(KD):
                po = ps_pr.tile([128, 512], FP, tag="pr")
                for k in range(KD):
                    nc.tensor.matmul(po[:], W["wo"][:, k, dm * 128:(dm + 1) * 128],
                                     hsT[:, k, B + h * 512:B + (h + 1) * 512],
                                     start=(k == 0), stop=(k == KD - 1))
                nc.vector.tensor_tensor(tdst[:, dm, cs], tsrc[:, dm, cs],
                                        po[:], ALU.add)

    # ================= rounds =================
    for r in range(c.R):
        # ---- slot A input from the ring ----
        if r >= 1:
            nc.sync.dma_start(inA[:],
                              obs[r - 1][bass.ds(off * 128, 128), :, :])
        # ---- embed cell, blended into inA (real on core 0 only; no real
        # chunk enters the pipeline after round NC-1) ----
        xcol = min(r, c.NC - 1) * CC
        if r < c.NC:
            x_row = sb.tile([1, CC], FP, tag="xrow", bufs=2)
            nc.sync.dma_start(x_row[:], ins["x_tb"][:, xcol:xcol + CC])
            for h in range(NH):
                cs = slice(h * 512, (h + 1) * 512)
                xb_ps = ps_bc.tile([128, 512], FP, tag="bc")
                nc.tensor.matmul(xb_ps[:], onec_f[:], x_row[:, cs],
                                 start=True, stop=True)
                ohs = []
                for vc in range(KV):
                    oh = sb.tile([128, 512], BF, tag=f"oh{vc}", bufs=2)
                    nc.vector.tensor_scalar(oh[:], xb_ps[:], iota2[:, vc:vc + 1],
                                            m_emb[:, 0:1], ALU.is_equal, ALU.mult)
                    ohs.append(oh)
                for dm in range(KD):
                    px = ps_pr.tile([128, 512], FP, tag="pr")
                    for vc in range(KV):
                        nc.tensor.matmul(px[:], e_sb[:, vc, dm * 128:(dm + 1) * 128],
                                         ohs[vc][:], start=(vc == 0),
                                         stop=(vc == KV - 1))
                    nc.vector.tensor_scalar(inA[:, dm, cs], inA[:, dm, cs],
                                            m_keep[:, 0:1], None, ALU.mult)
                    nc.vector.tensor_tensor(inA[:, dm, cs], inA[:, dm, cs], px[:],
                                            ALU.add)
        # ---- the two layer cells (serial within the round) ----
        cell(inA, Aout, WA, stA, m_st[:, r:r + 1])
        cell(Aout, Bout, WB, stB, m_st[:, r:r + 1])
        # ---- logits cell (real on core 5 from round 5 on) ----
        for h in (range(NH) if r >= 5 else []):
            hnF = sb.tile([128, KD, 512], BF, tag="hn", bufs=3)
            layer_norm_half(Bout, h, hnF)
            for t4 in range(4):
                pl = ps_pr.tile([128, c.V], FP, tag="pr")
                for k in range(KD):
                    nc.tensor.matmul(pl[:], hnF[:, k, t4 * 128:(t4 + 1) * 128],
                                     et_sb[:, k, :], start=(k == 0), stop=False)
                nc.tensor.matmul(pl[:], onec_b[:], bv_sb[:], start=False,
                                 stop=True)
                out_sb = sb.tile([128, c.V], FP, tag="osb", bufs=3)
                nc.vector.tensor_copy(out_sb[:], pl[:])
                r0 = r * CC + h * 512 + t4 * 128
                nc.sync.dma_start(lg_out[r0:r0 + 128, :], out_sb[:])
        # ---- payload + AllGather (the last round's AG has no consumer) ----
        if r < c.R - 1:
            ib = dram.tile([128, KD, CC], BF, tag="ib")
            nc.sync.dma_start(ib[:], Bout[:])
            if not NO_COLLECTIVES:
                nc.gpsimd.collective_compute(
                    "AllGather", ALU.bypass,
                    replica_groups=[list(range(c.n_cores))],
                    ins=[ib.opt()], outs=[obs[r].opt()],
                )
            else:
                nc.sync.dma_start(obs[r][0:128, :, :], ib[:])


# ======================= host side =======================

def _pack_lhsT(m, kchunks, dtype):
    K, J = m.shape
    assert K == kchunks * 128
    return np.ascontiguousarray(m.reshape(kchunks, 128, J).transpose(1, 0, 2),
                                dtype=dtype)


def prep_inputs(inputs, cfg: "Cfg"):
    import ml_dtypes
    bf = ml_dtypes.bfloat16
    c = cfg
    f8 = np.float64
    x = np.asarray(inputs["x"])
    emb = np.asarray(inputs["embedding"], f8)
    ln_g = np.asarray(inputs["ln_gamma"], f8)
    ln_b = np.asarray(inputs["ln_beta"], f8)
    Win = np.asarray(inputs["Win"], f8)
    W_zr = np.asarray(inputs["W_zr"], f8)
    U_zr = np.asarray(inputs["U_zr"], f8)
    W_h = np.asarray(inputs["W_h"], f8)
    U_h = np.asarray(inputs["U_h"], f8)
    b_zr = np.asarray(inputs["b_zr"], f8)
    b_h = np.asarray(inputs["b_h"], f8)
    Wout = np.asarray(inputs["Wout"], f8)
    ng = np.asarray(inputs["norm_gamma"], f8)
    nb = np.asarray(inputs["norm_beta"], f8)

    def layer_pack(l):
        if l >= c.DEPTH:
            z2 = np.zeros((128, c.KD, 2 * c.DI), bf)
            z1 = np.zeros((128, c.KD, c.DI), bf)
            return dict(uzrT=z2, uhT=z1, wzrT=z2.copy(), whT=z1.copy(),
                        woT=np.zeros((128, c.KD, c.D), bf),
                        bzr=np.zeros((128, c.MZR), np.float32),
                        bh=np.zeros((128, c.MH), np.float32))
        Wzr_eff = W_zr[l] @ Win[l]
        bzr_eff = Wzr_eff @ ln_b[l] + b_zr[l]
        Wzr_eff = Wzr_eff * ln_g[l][None, :]
        Wh_eff = W_h[l] @ Win[l]
        bh_eff = Wh_eff @ ln_b[l] + b_h[l]
        Wh_eff = Wh_eff * ln_g[l][None, :]
        return dict(
            uzrT=_pack_lhsT(U_zr[l].T, c.KD, bf),
            uhT=_pack_lhsT(U_h[l].T, c.KD, bf),
            wzrT=_pack_lhsT(Wzr_eff.T, c.KD, bf),
            whT=_pack_lhsT(Wh_eff.T, c.KD, bf),
            woT=_pack_lhsT(Wout[l].T, c.KD, bf),
            bzr=np.ascontiguousarray(bzr_eff.reshape(c.MZR, 128).T, np.float32),
            bh=np.ascontiguousarray(bh_eff.reshape(c.MH, 128).T, np.float32),
        )

    shared = {}
    shared["E_lhsT"] = np.ascontiguousarray(
        emb.reshape(c.KV, 128, c.D).transpose(1, 0, 2), dtype=bf)
    shared["ET_rhs"] = _pack_lhsT((emb * ng[None, :]).T, c.KD, bf)
    shared["bv_row"] = np.ascontiguousarray((emb @ nb)[None, :], dtype=bf)
    shared["iota2"] = np.ascontiguousarray(
        (np.arange(128)[:, None] + 128 * np.arange(c.KV)[None, :]), np.float32)
    shared["ones_col"] = np.ones((1, 128), np.float32)
    shared["ones_k"] = np.ones((128, 1), np.float32)
    shared["x_tb"] = np.ascontiguousarray(x.T.reshape(1, -1), dtype=np.float32)

    in_maps = []
    for core in range(c.n_cores):
        m = dict(shared)
        la = 2 * core if core < 6 else c.DEPTH       # >= DEPTH -> zero weights
        lb = 2 * core + 1 if core < 6 else c.DEPTH
        for kk, v in layer_pack(la).items():
            m["LA_" + kk] = v
        for kk, v in layer_pack(lb).items():
            m["LB_" + kk] = v
        mst = np.zeros((128, c.R), np.float32)
        if core < 6:
            for r in range(c.R):
                if core < r < core + c.NC:
                    mst[:, r] = 1.0
        m["m_st"] = mst
        m["m_emb"] = np.full((128, 1), 1.0 if core == 0 else 0.0, np.float32)
        m["m_keep"] = np.full((128, 1), 0.0 if core == 0 else 1.0, np.float32)
        m["prev_off"] = np.array([[(core - 1) % c.n_cores]], np.uint32)
        in_maps.append(m)
    return in_maps, shared


def declare_tensors(nc, cfg: "Cfg", in_map0):
    c = cfg
    ins = {}
    for name, arr in in_map0.items():
        dt = mybir.dt.from_np(arr.dtype)
        ins[name] = nc.dram_tensor(name, list(arr.shape), dt,
                                   kind="ExternalInput").ap()
    outs = {}
    outs["logits"] = nc.dram_tensor("logits", [c.R * c.CC, c.V], FP,
                                    kind="ExternalOutput").ap()
    return outs, ins


_CACHE = {}


def build_program(cfg: "Cfg", in_map0):
    key = (cfg.DEPTH, cfg.T, cfg.S, cfg.U, cfg.n_cores, NO_COLLECTIVES,
           UNROLL_SCAN, SCAN_MM_ONLY)
    if key in _CACHE:
        return _CACHE[key]
    nc = bacc.Bacc("TRN2", target_bir_lowering=False, debug=False,
                   num_devices=cfg.n_cores)
    outs, ins = declare_tensors(nc, cfg, in_map0)
    with tile.TileContext(nc) as tc:
        with ExitStack() as ctx:
            shp = ctx.enter_context(tc.tile_pool(name="shdram", bufs=1,
                                                 space="DRAM"))
            obs = [shp.tile([cfg.n_cores * 128, cfg.KD, cfg.CC], BF,
                            addr_space="Shared", name=f"ob{r}")
                   for r in range(cfg.R)]
            build_kernel(ctx, tc, outs, ins, obs, cfg)
    nc.compile()
    _CACHE[key] = nc
    return nc


def kernel(**inputs) -> np.ndarray:
    cfg = Cfg()
    in_maps, shared = prep_inputs(inputs, cfg)
    nc = build_program(cfg, in_maps[0])
    res = run_bass_kernel_spmd(nc, in_maps, core_ids=list(range(cfg.n_cores)))
    lg = res.results[5]["logits"]          # [R*CC, V]
    c = cfg
    out = np.empty((c.B, c.T, c.V), np.float32)
    for ch in range(c.NC):
        blk = lg[(5 + ch) * c.CC:(6 + ch) * c.CC, :]
        blk = blk.reshape(c.S, c.B, c.V).transpose(1, 0, 2)
        out[:, ch * c.S:(ch + 1) * c.S, :] = blk
    return np.ascontiguousarray(out)


if __name__ == "__main__":
    rng = np.random.default_rng(0)
    ins = dict(
        x=rng.integers(0, 256, size=(16, 2048)),
        embedding=rng.normal(size=(256, 512)).astype(np.float32) * 0.02,
        ln_gamma=np.ones((12, 512), np.float32),
        ln_beta=np.zeros((12, 512), np.float32),
        Win=rng.normal(size=(12, 512, 512)).astype(np.float32) * 0.02,
        W_zr=rng.normal(size=(12, 1024, 512)).astype(np.float32) * 0.02,
        U_zr=rng.normal(size=(12, 1024, 512)).astype(np.float32) * 0.04,
        W_h=rng.normal(size=(12, 512, 512)).astype(np.float32) * 0.02,
        U_h=rng.normal(size=(12, 512, 512)).astype(np.float32) * 0.04,
        b_zr=np.zeros((12, 1024), np.float32),
        b_h=np.zeros((12, 512), np.float32),
        Wout=rng.normal(size=(12, 512, 512)).astype(np.float32) * 0.02,
        norm_gamma=np.ones((512,), np.float32),
        norm_beta=np.zeros((512,), np.float32),
    )
    out = kernel(**ins)
    print(out.shape, out.dtype, np.abs(out).max())



# revision 17
# speedup vs baseline: 1.9544x; 1.9544x over previous
"""Trainium2 Bass kernel for nn_CudaGRULM — cross-core layer pipeline.

The GRU scan is LDWEIGHTS-bound (48 128x128 weight tiles per step). The
data-parallel baseline gives each matmul only B/8 = 2 batch columns; here we
pipeline LAYERS across cores with the FULL batch B=16 per step: core k hosts
layers 2k and 2k+1 (k = 0..5); a chunk of S timesteps passes through both
layers within one round, then hops to core k+1 via one AllGather per round.
Core 0 folds the embedding into its input (masked one-hot matmul); core 5
computes the final LN + logits. Cores 6, 7 run the same SPMD program on zero
weights. Inside the scan, r-gates are computed first so the z-gate matmuls
fill the PE while the r-chain runs on DVE/ACT.

The scan is fully python-unrolled: matmuls whose operands carry
register-offset access patterns (For_i loop variable) cost ~195ns each on HW
vs ~40-60ns with static APs, so static unrolling is worth the program size.
Rounds are python-unrolled (collectives cannot live inside hardware loops);
each round's AllGather writes its own Shared DRAM buffer (single-writer
rule). Garbage cells at pipeline fill/drain stay finite by induction (all
buffers memset once; round 0 skips the AG read), and per-(core,round) state
masks zero the scan state at each layer's first real chunk.
"""

from contextlib import ExitStack

import numpy as np

import concourse.bass as bass
import concourse.bacc as bacc
import concourse.tile as tile
from concourse import mybir
from concourse.bass_utils import run_bass_kernel_spmd

FP = mybir.dt.float32
BF = mybir.dt.bfloat16
AF = mybir.ActivationFunctionType
ALU = mybir.AluOpType

# test/experiment knobs (kernel() itself always runs the default config):
# NO_COLLECTIVES replaces the AllGather with a local DRAM copy (wrong output,
# used for timing A/B and TimelineSim); UNROLL_SCAN statically unrolls the
# scan loop (needed by TimelineSim, which has no register interpreter).
NO_COLLECTIVES = False
UNROLL_SCAN = True
# timing diagnostic: keep only the scan's matmuls (drop the DVE/ACT chain);
# output is garbage — used to separate PE stream cost from chain stalls
SCAN_MM_ONLY = False


class Cfg:
    def __init__(self, V=256, D=512, DEPTH=12, DI=512, B=16, T=2048,
                 n_cores=8, S=64, U=8, EPS=1e-5):
        self.V, self.D, self.DEPTH, self.DI, self.B, self.T = V, D, DEPTH, DI, B, T
        self.n_cores = n_cores
        self.S = S                      # scan steps per chunk
        self.U = U                      # scan unroll inside For_i
        self.NC = T // S                # chunks
        self.CC = S * B                 # chunk cols (token-major: col = t*B+b)
        self.EPS = EPS
        self.KD = D // 128              # 4
        self.KV = V // 128              # 2
        self.MZR = 2 * DI // 128        # 8
        self.MH = DI // 128             # 4
        self.NH = self.CC // 512        # 512-col groups per chunk
        # core k processes chunk c through layers 2k,2k+1 at round c+k;
        # logits on core 5 in the same round -> last round = (NC-1)+5
        self.R = self.NC + 5
        assert D == DI and self.CC % 512 == 0 and S % U == 0


def build_kernel(ctx: ExitStack, tc: "tile.TileContext", outs, ins, obs, cfg: Cfg):
    nc = tc.nc
    c = cfg
    KD, KV, MZR, MH, B, CC, S, U, NH = (c.KD, c.KV, c.MZR, c.MH, c.B, c.CC,
                                        c.S, c.U, c.NH)
    MR = MZR // 2
    lg_out = outs["logits"]

    persist = ctx.enter_context(tc.tile_pool(name="persist", bufs=1))
    sb = ctx.enter_context(tc.tile_pool(name="sb", bufs=2))
    dram = ctx.enter_context(tc.tile_pool(name="dram", bufs=2, space="DRAM"))
    # scan psum: r/z/h groups in SEPARATE banks so PE writes of one group
    # never bank-collide with ACT/DVE reads of another (Tile's bank-aware
    # tracker would otherwise serialize the whole step around one bank)
    ps_r = ctx.enter_context(tc.tile_pool(name="ps_r", bufs=2, space="PSUM"))
    ps_z = ctx.enter_context(tc.tile_pool(name="ps_z", bufs=2, space="PSUM"))
    ps_h = ctx.enter_context(tc.tile_pool(name="ps_h", bufs=2, space="PSUM"))
    ps_pr = ctx.enter_context(tc.tile_pool(name="ps_pr", bufs=2, space="PSUM"))

    # ---- constants ----
    iota2 = persist.tile([128, KV], FP)
    nc.sync.dma_start(iota2[:], ins["iota2"][:])
    onec_f = persist.tile([1, 128], FP)
    nc.sync.dma_start(onec_f[:], ins["ones_col"][:])
    onec_b = persist.tile([1, 128], BF)
    nc.vector.tensor_copy(onec_b[:], onec_f[:])
    onek_f = persist.tile([128, 1], FP)
    nc.sync.dma_start(onek_f[:], ins["ones_k"][:])
    onek_b = persist.tile([128, 1], BF)
    nc.vector.tensor_copy(onek_b[:], onek_f[:])
    e_sb = persist.tile([128, KV, c.D], BF)
    nc.sync.dma_start(e_sb[:], ins["E_lhsT"][:])
    et_sb = persist.tile([128, KD, c.V], BF)
    nc.sync.dma_start(et_sb[:], ins["ET_rhs"][:])
    bv_sb = persist.tile([1, c.V], BF)
    nc.sync.dma_start(bv_sb[:], ins["bv_row"][:])
    eps_sb = persist.tile([1, 1], FP)
    nc.vector.memset(eps_sb[:], float(c.EPS))
    m_emb = persist.tile([128, 1], FP)
    nc.sync.dma_start(m_emb[:], ins["m_emb"][:])
    m_keep = persist.tile([128, 1], FP)
    nc.sync.dma_start(m_keep[:], ins["m_keep"][:])
    m_st = persist.tile([128, c.R], FP)
    nc.sync.dma_start(m_st[:], ins["m_st"][:])
    ident = persist.tile([128, 128], BF)
    nc.sync.dma_start(ident[:], ins["ident"][:])

    # ---- per-slot weights (resident the whole run) ----
    def load_w(pfx):
        shapes = dict(uzr=([128, KD, 2 * c.DI], BF), uh=([128, KD, c.DI], BF),
                      wzr=([128, KD, 2 * c.DI], BF), wh=([128, KD, c.DI], BF),
                      wo=([128, KD, c.D], BF), bzr=([128, MZR], FP),
                      bh=([128, MH], FP))
        srcs = dict(uzr="uzrT", uh="uhT", wzr="wzrT", wh="whT", wo="woT",
                    bzr="bzr", bh="bh")
        w = {}
        for kk, (shp_, dt_) in shapes.items():
            w[kk] = persist.tile(shp_, dt_, name=pfx + kk)
            nc.sync.dma_start(w[kk][:], ins[pfx + srcs[kk]][:])
        return w

    WA = load_w("LA_")
    WB = load_w("LB_")

    # ---- streaming buffers (cells are serial within a round -> shared) ----
    inA = persist.tile([128, KD, CC], BF)      # slot A input chunk
    Aout = persist.tile([128, KD, CC], BF)     # layer-2k output
    Bout = persist.tile([128, KD, CC], BF)     # layer-2k+1 output (payload)
    stA = persist.tile([128, KD, B], FP)       # carried scan states
    stB = persist.tile([128, KD, B], FP)
    hsT = persist.tile([128, KD, (S + 1) * B], BF)   # scan out ring (shared)
    xzr = persist.tile([128, MZR, CC], BF)           # input proj (shared)
    xh = persist.tile([128, MH, CC], BF)
    nc.vector.memset(inA[:], 0.0)
    nc.vector.memset(Aout[:], 0.0)
    nc.vector.memset(Bout[:], 0.0)
    nc.vector.memset(stA[:], 0.0)
    nc.vector.memset(stB[:], 0.0)

    # per-core row offset of the previous rank inside AG outputs
    if NO_COLLECTIVES:
        off = c.n_cores - 1
    else:
        off_reg = nc.sync.alloc_register("prev_off")
        nc.sync.reg_load(off_reg, ins["prev_off"][0:1, 0:1])
        off = nc.sync.snap(off_reg, donate=True, min_val=0, max_val=c.n_cores - 1)

    def layer_norm_half(tsrc, h, hn):
        """LN of cols [h*512,(h+1)*512) of tsrc ([128,KD,CC] bf16) into hn."""
        cs = slice(h * 512, (h + 1) * 512)
        mean_ps = ps_pr.tile([128, 512], FP, tag="pr")
        for k in range(KD):
            nc.tensor.matmul(mean_ps[0:1, :], onek_b[:], tsrc[:, k, cs],
                             start=(k == 0), stop=(k == KD - 1))
        sq = sb.tile([128, KD, 512], BF, tag="sq", bufs=2)
        for k in range(KD):
            nc.scalar.activation(sq[:, k, :], tsrc[:, k, cs], AF.Square)
        mean_r = sb.tile([1, 512], FP, tag="lnrow", bufs=8)
        nc.vector.tensor_scalar(mean_r[:], mean_ps[0:1, :], 1.0 / c.D, None,
                                ALU.mult)
        sq_ps = ps_pr.tile([128, 512], FP, tag="pr")
        for k in range(KD):
            nc.tensor.matmul(sq_ps[0:1, :], onek_b[:], sq[:, k, :],
                             start=(k == 0), stop=(k == KD - 1))
        msq_r = sb.tile([1, 512], FP, tag="lnrow", bufs=8)
        nc.vector.tensor_scalar(msq_r[:], sq_ps[0:1, :], 1.0 / c.D, None,
                                ALU.mult)
        var_r = sb.tile([1, 512], FP, tag="lnrow", bufs=8)
        nc.vector.tensor_tensor(var_r[:], mean_r[:], mean_r[:], ALU.mult)
        nc.vector.tensor_tensor(var_r[:], msq_r[:], var_r[:], ALU.subtract)
        std_r = sb.tile([1, 512], FP, tag="lnrow", bufs=8)
        nc.scalar.activation(std_r[:], var_r[:], AF.Sqrt, bias=eps_sb[:])
        rstd_r = sb.tile([1, 512], FP, tag="lnrow", bufs=8)
        nc.vector.reciprocal(rstd_r[:], std_r[:])
        mr_r = sb.tile([1, 512], FP, tag="lnrow", bufs=8)
        nc.vector.tensor_tensor(mr_r[:], mean_r[:], rstd_r[:], ALU.mult)
        rb_ps = ps_pr.tile([128, 512], FP, tag="pr")
        nc.tensor.matmul(rb_ps[:], onec_f[:], rstd_r[:], start=True, stop=True)
        mrb_ps = ps_pr.tile([128, 512], FP, tag="pr")
        nc.tensor.matmul(mrb_ps[:], onec_f[:], mr_r[:], start=True, stop=True)
        for k in range(KD):
            nc.vector.tensor_tensor(hn[:, k, :], tsrc[:, k, cs], rb_ps[:],
                                    ALU.mult)
            nc.vector.tensor_tensor(hn[:, k, :], hn[:, k, :], mrb_ps[:],
                                    ALU.subtract)

    def cell(tsrc, tdst, W, st, mask_col):
        # 1. masked carried state -> step-0 state (bf16 ring col 0)
        nc.vector.tensor_scalar(st[:], st[:], mask_col, None, ALU.mult)
        nc.vector.tensor_copy(hsT[:, :, 0:B], st[:])
        # 2. LN + input projections, per 512-col group
        for h in range(NH):
            cs = slice(h * 512, (h + 1) * 512)
            hn = sb.tile([128, KD, 512], BF, tag="hn", bufs=3)
            layer_norm_half(tsrc, h, hn)
            for m in range(MZR):
                px = ps_pr.tile([128, 512], FP, tag="pr")
                for k in range(KD):
                    nc.tensor.matmul(px[:], W["wzr"][:, k, m * 128:(m + 1) * 128],
                                     hn[:, k, :], start=(k == 0),
                                     stop=(k == KD - 1))
                nc.scalar.activation(xzr[:, m, cs], px[:], AF.Identity,
                                     bias=W["bzr"][:, m:m + 1])
            for m in range(MH):
                px = ps_pr.tile([128, 512], FP, tag="pr")
                for k in range(KD):
                    nc.tensor.matmul(px[:], W["wh"][:, k, m * 128:(m + 1) * 128],
                                     hn[:, k, :], start=(k == 0),
                                     stop=(k == KD - 1))
                nc.scalar.activation(xh[:, m, cs], px[:], AF.Identity,
                                     bias=W["bh"][:, m:m + 1])
        # 3. the scan: r gates first; z matmuls fill PE during the r chain
        def scan_trip(it):
            prev = None
            for u in range(U):
                cin = bass.ds((it + u) * B, B)
                cout = bass.ds((it + u + 1) * B, B)
                if prev is None:
                    prev = st
                rps = ps_r.tile([128, MH * B], FP, tag="r")
                zps = ps_z.tile([128, MR * B], FP, tag="z")
                hps = ps_h.tile([128, MH * B], FP, tag="h")
                # prefold x-projections into psum via identity matmuls: the
                # first clears the bank (start=True); the rest overwrite
                # their own has_written-clear slots. The sigmoid/tanh then
                # read psum directly — no DVE add on the critical path.
                for m in range(MH):
                    nc.tensor.matmul(rps[:, m * B:(m + 1) * B], ident[:],
                                     xzr[:, MR + m, cin], start=(m == 0),
                                     stop=False, skip_group_check=True)
                for m in range(MH):
                    nc.tensor.matmul(hps[:, m * B:(m + 1) * B], ident[:],
                                     xh[:, m, cin], start=(m == 0),
                                     stop=False, skip_group_check=True)
                # r gates (gated on the previous step's ring write)
                for m in range(MR, MZR):
                    for k in range(KD):
                        nc.tensor.matmul(rps[:, (m - MR) * B:(m - MR + 1) * B],
                                         W["uzr"][:, k, m * 128:(m + 1) * 128],
                                         hsT[:, k, cin], start=False,
                                         stop=(m == MZR - 1 and k == KD - 1),
                                         skip_group_check=True)
                # z gates
                for m in range(MR):
                    for k in range(KD):
                        nc.tensor.matmul(zps[:, m * B:(m + 1) * B],
                                         W["uzr"][:, k, m * 128:(m + 1) * 128],
                                         hsT[:, k, cin],
                                         start=(k == 0), stop=(k == KD - 1))
                if SCAN_MM_ONLY:
                    for m in range(MH):
                        for k in range(KD):
                            nc.tensor.matmul(hps[:, m * B:(m + 1) * B],
                                             W["uh"][:, k, m * 128:(m + 1) * 128],
                                             hsT[:, k, cin], start=False,
                                             stop=(m == MH - 1 and k == KD - 1),
                                             skip_group_check=True)
                    prev = st
                    continue
                # r chain: sigmoid straight off psum; rh in bf16 for the MM
                za_r = sb.tile([128, MH, B], FP, tag="szar", bufs=3)
                nc.scalar.activation(za_r[:], rps[:], AF.Sigmoid)
                rh = sb.tile([128, KD, B], BF, tag="srh", bufs=3)
                nc.vector.tensor_tensor(rh[:], za_r[:], prev[:], ALU.mult)
                # z chain (off the critical path): z and 1-z = sigmoid(-x);
                # a = (1-z)*prev precomputed before the candidate lands
                zs_z = sb.tile([128, MR, B], FP, tag="szsz", bufs=3)
                nc.vector.tensor_tensor(zs_z[:], zps[:], xzr[:, 0:MR, cin],
                                        ALU.add)
                za_z = sb.tile([128, MR, B], FP, tag="szaz", bufs=3)
                nc.scalar.activation(za_z[:], zs_z[:], AF.Sigmoid)
                zc = sb.tile([128, MR, B], FP, tag="szc", bufs=3)
                nc.scalar.activation(zc[:], zs_z[:], AF.Sigmoid, scale=-1.0)
                a = sb.tile([128, KD, B], FP, tag="sa", bufs=3)
                nc.gpsimd.tensor_tensor(a[:], zc[:], prev[:], ALU.mult)
                # candidate (gated by rh)
                for m in range(MH):
                    for k in range(KD):
                        nc.tensor.matmul(hps[:, m * B:(m + 1) * B],
                                         W["uh"][:, k, m * 128:(m + 1) * 128],
                                         rh[:, k, :], start=False,
                                         stop=(m == MH - 1 and k == KD - 1),
                                         skip_group_check=True)
                hc = sb.tile([128, MH, B], FP, tag="shc", bufs=3)
                nc.scalar.activation(hc[:], hps[:], AF.Tanh)
                b = sb.tile([128, KD, B], FP, tag="sb2", bufs=3)
                nc.vector.tensor_tensor(b[:], za_z[:], hc[:], ALU.mult)
                # h_t = a + b: bf16 ring write on DVE (critical path), fp32
                # carried state on gpsimd in parallel (off-path)
                nc.vector.tensor_tensor(hsT[:, :, cout], a[:], b[:], ALU.add)
                nxt = sb.tile([128, KD, B], FP, tag="snx", bufs=3)
                nc.gpsimd.tensor_tensor(nxt[:], a[:], b[:], ALU.add)
                prev = nxt
            # carry fp32 state across trips / rounds
            if not SCAN_MM_ONLY:
                nc.vector.tensor_copy(st[:], prev[:])

        if UNROLL_SCAN:
            for it0 in range(0, S, U):
                scan_trip(it0)
        else:
            with tc.For_i(0, S, U, hint_engines=(mybir.EngineType.PE,)) as it:
                scan_trip(it)
        # 4. output projection + residual
        for h in range(NH):
            cs = slice(h * 512, (h + 1) * 512)
            for dm in range(KD):
                po = ps_pr.tile([128, 512], FP, tag="pr")
                for k in range(KD):
                    nc.tensor.matmul(po[:], W["wo"][:, k, dm * 128:(dm + 1) * 128],
                                     hsT[:, k, B + h * 512:B + (h + 1) * 512],
                                     start=(k == 0), stop=(k == KD - 1))
                nc.vector.tensor_tensor(tdst[:, dm, cs], tsrc[:, dm, cs],
                                        po[:], ALU.add)

    # ================= rounds =================
    for r in range(c.R):
        # ---- slot A input from the ring ----
        if r >= 1:
            nc.sync.dma_start(inA[:],
                              obs[r - 1][bass.ds(off * 128, 128), :, :])
        # ---- embed cell, blended into inA (real on core 0 only; no real
        # chunk enters the pipeline after round NC-1) ----
        xcol = min(r, c.NC - 1) * CC
        if r < c.NC:
            x_row = sb.tile([1, CC], FP, tag="xrow", bufs=2)
            nc.sync.dma_start(x_row[:], ins["x_tb"][:, xcol:xcol + CC])
            for h in range(NH):
                cs = slice(h * 512, (h + 1) * 512)
                xb_ps = ps_pr.tile([128, 512], FP, tag="pr")
                nc.tensor.matmul(xb_ps[:], onec_f[:], x_row[:, cs],
                                 start=True, stop=True)
                ohs = []
                for vc in range(KV):
                    oh = sb.tile([128, 512], BF, tag=f"oh{vc}", bufs=2)
                    nc.vector.tensor_scalar(oh[:], xb_ps[:], iota2[:, vc:vc + 1],
                                            m_emb[:, 0:1], ALU.is_equal, ALU.mult)
                    ohs.append(oh)
                for dm in range(KD):
                    px = ps_pr.tile([128, 512], FP, tag="pr")
                    for vc in range(KV):
                        nc.tensor.matmul(px[:], e_sb[:, vc, dm * 128:(dm + 1) * 128],
                                         ohs[vc][:], start=(vc == 0),
                                         stop=(vc == KV - 1))
                    nc.vector.tensor_scalar(inA[:, dm, cs], inA[:, dm, cs],
                                            m_keep[:, 0:1], None, ALU.mult)
                    nc.vector.tensor_tensor(inA[:, dm, cs], inA[:, dm, cs], px[:],
                                            ALU.add)
        # ---- the two layer cells (serial within the round) ----
        cell(inA, Aout, WA, stA, m_st[:, r:r + 1])
        cell(Aout, Bout, WB, stB, m_st[:, r:r + 1])
        # ---- logits cell (real on core 5 from round 5 on) ----
        for h in (range(NH) if r >= 5 else []):
            hnF = sb.tile([128, KD, 512], BF, tag="hn", bufs=3)
            layer_norm_half(Bout, h, hnF)
            for t4 in range(4):
                pl = ps_pr.tile([128, c.V], FP, tag="pr")
                for k in range(KD):
                    nc.tensor.matmul(pl[:], hnF[:, k, t4 * 128:(t4 + 1) * 128],
                                     et_sb[:, k, :], start=(k == 0), stop=False)
                nc.tensor.matmul(pl[:], onec_b[:], bv_sb[:], start=False,
                                 stop=True)
                out_sb = sb.tile([128, c.V], FP, tag="osb", bufs=3)
                nc.vector.tensor_copy(out_sb[:], pl[:])
                r0 = r * CC + h * 512 + t4 * 128
                nc.sync.dma_start(lg_out[r0:r0 + 128, :], out_sb[:])
        # ---- payload + AllGather (the last round's AG has no consumer) ----
        if r < c.R - 1:
            ib = dram.tile([128, KD, CC], BF, tag="ib")
            nc.sync.dma_start(ib[:], Bout[:])
            if not NO_COLLECTIVES:
                nc.gpsimd.collective_compute(
                    "AllGather", ALU.bypass,
                    replica_groups=[list(range(c.n_cores))],
                    ins=[ib.opt()], outs=[obs[r].opt()],
                )
            else:
                nc.sync.dma_start(obs[r][0:128, :, :], ib[:])


# ======================= host side =======================

def _pack_lhsT(m, kchunks, dtype):
    K, J = m.shape
    assert K == kchunks * 128
    return np.ascontiguousarray(m.reshape(kchunks, 128, J).transpose(1, 0, 2),
                                dtype=dtype)


def prep_inputs(inputs, cfg: "Cfg"):
    import ml_dtypes
    bf = ml_dtypes.bfloat16
    c = cfg
    f8 = np.float64
    x = np.asarray(inputs["x"])
    emb = np.asarray(inputs["embedding"], f8)
    ln_g = np.asarray(inputs["ln_gamma"], f8)
    ln_b = np.asarray(inputs["ln_beta"], f8)
    Win = np.asarray(inputs["Win"], f8)
    W_zr = np.asarray(inputs["W_zr"], f8)
    U_zr = np.asarray(inputs["U_zr"], f8)
    W_h = np.asarray(inputs["W_h"], f8)
    U_h = np.asarray(inputs["U_h"], f8)
    b_zr = np.asarray(inputs["b_zr"], f8)
    b_h = np.asarray(inputs["b_h"], f8)
    Wout = np.asarray(inputs["Wout"], f8)
    ng = np.asarray(inputs["norm_gamma"], f8)
    nb = np.asarray(inputs["norm_beta"], f8)

    def layer_pack(l):
        if l >= c.DEPTH:
            z2 = np.zeros((128, c.KD, 2 * c.DI), bf)
            z1 = np.zeros((128, c.KD, c.DI), bf)
            return dict(uzrT=z2, uhT=z1, wzrT=z2.copy(), whT=z1.copy(),
                        woT=np.zeros((128, c.KD, c.D), bf),
                        bzr=np.zeros((128, c.MZR), np.float32),
                        bh=np.zeros((128, c.MH), np.float32))
        Wzr_eff = W_zr[l] @ Win[l]
        bzr_eff = Wzr_eff @ ln_b[l] + b_zr[l]
        Wzr_eff = Wzr_eff * ln_g[l][None, :]
        Wh_eff = W_h[l] @ Win[l]
        bh_eff = Wh_eff @ ln_b[l] + b_h[l]
        Wh_eff = Wh_eff * ln_g[l][None, :]
        return dict(
            uzrT=_pack_lhsT(U_zr[l].T, c.KD, bf),
            uhT=_pack_lhsT(U_h[l].T, c.KD, bf),
            wzrT=_pack_lhsT(Wzr_eff.T, c.KD, bf),
            whT=_pack_lhsT(Wh_eff.T, c.KD, bf),
            woT=_pack_lhsT(Wout[l].T, c.KD, bf),
            bzr=np.ascontiguousarray(bzr_eff.reshape(c.MZR, 128).T, np.float32),
            bh=np.ascontiguousarray(bh_eff.reshape(c.MH, 128).T, np.float32),
        )

    shared = {}
    shared["E_lhsT"] = np.ascontiguousarray(
        emb.reshape(c.KV, 128, c.D).transpose(1, 0, 2), dtype=bf)
    shared["ET_rhs"] = _pack_lhsT((emb * ng[None, :]).T, c.KD, bf)
    shared["bv_row"] = np.ascontiguousarray((emb @ nb)[None, :], dtype=bf)
    shared["iota2"] = np.ascontiguousarray(
        (np.arange(128)[:, None] + 128 * np.arange(c.KV)[None, :]), np.float32)
    shared["ones_col"] = np.ones((1, 128), np.float32)
    shared["ones_k"] = np.ones((128, 1), np.float32)
    shared["ident"] = np.eye(128, dtype=bf)
    shared["x_tb"] = np.ascontiguousarray(x.T.reshape(1, -1), dtype=np.float32)

    in_maps = []
    for core in range(c.n_cores):
        m = dict(shared)
        la = 2 * core if core < 6 else c.DEPTH       # >= DEPTH -> zero weights
        lb = 2 * core + 1 if core < 6 else c.DEPTH
        for kk, v in layer_pack(la).items():
            m["LA_" + kk] = v
        for kk, v in layer_pack(lb).items():
            m["LB_" + kk] = v
        mst = np.zeros((128, c.R), np.float32)
        if core < 6:
            for r in range(c.R):
                if core < r < core + c.NC:
                    mst[:, r] = 1.0
        m["m_st"] = mst
        m["m_emb"] = np.full((128, 1), 1.0 if core == 0 else 0.0, np.float32)
        m["m_keep"] = np.full((128, 1), 0.0 if core == 0 else 1.0, np.float32)
        m["prev_off"] = np.array([[(core - 1) % c.n_cores]], np.uint32)
        in_maps.append(m)
    return in_maps, shared


def declare_tensors(nc, cfg: "Cfg", in_map0):
    c = cfg
    ins = {}
    for name, arr in in_map0.items():
        dt = mybir.dt.from_np(arr.dtype)
        ins[name] = nc.dram_tensor(name, list(arr.shape), dt,
                                   kind="ExternalInput").ap()
    outs = {}
    outs["logits"] = nc.dram_tensor("logits", [c.R * c.CC, c.V], FP,
                                    kind="ExternalOutput").ap()
    return outs, ins


_CACHE = {}


def build_program(cfg: "Cfg", in_map0):
    key = (cfg.DEPTH, cfg.T, cfg.S, cfg.U, cfg.n_cores, NO_COLLECTIVES,
           UNROLL_SCAN, SCAN_MM_ONLY)
    if key in _CACHE:
        return _CACHE[key]
    nc = bacc.Bacc("TRN2", target_bir_lowering=False, debug=False,
                   num_devices=cfg.n_cores)
    outs, ins = declare_tensors(nc, cfg, in_map0)
    with tile.TileContext(nc) as tc:
        with ExitStack() as ctx:
            shp = ctx.enter_context(tc.tile_pool(name="shdram", bufs=1,
                                                 space="DRAM"))
            obs = [shp.tile([cfg.n_cores * 128, cfg.KD, cfg.CC], BF,
                            addr_space="Shared", name=f"ob{r}")
                   for r in range(cfg.R)]
            build_kernel(ctx, tc, outs, ins, obs, cfg)
    nc.compile()
    _CACHE[key] = nc
    return nc


def kernel(**inputs) -> np.ndarray:
    cfg = Cfg()
    in_maps, shared = prep_inputs(inputs, cfg)
    nc = build_program(cfg, in_maps[0])
    res = run_bass_kernel_spmd(nc, in_maps, core_ids=list(range(cfg.n_cores)))
    lg = res.results[5]["logits"]          # [R*CC, V]
    c = cfg
    out = np.empty((c.B, c.T, c.V), np.float32)
    for ch in range(c.NC):
        blk = lg[(5 + ch) * c.CC:(6 + ch) * c.CC, :]
        blk = blk.reshape(c.S, c.B, c.V).transpose(1, 0, 2)
        out[:, ch * c.S:(ch + 1) * c.S, :] = blk
    return np.ascontiguousarray(out)


if __name__ == "__main__":
    rng = np.random.default_rng(0)
    ins = dict(
        x=rng.integers(0, 256, size=(16, 2048)),
        embedding=rng.normal(size=(256, 512)).astype(np.float32) * 0.02,
        ln_gamma=np.ones((12, 512), np.float32),
        ln_beta=np.zeros((12, 512), np.float32),
        Win=rng.normal(size=(12, 512, 512)).astype(np.float32) * 0.02,
        W_zr=rng.normal(size=(12, 1024, 512)).astype(np.float32) * 0.02,
        U_zr=rng.normal(size=(12, 1024, 512)).astype(np.float32) * 0.04,
        W_h=rng.normal(size=(12, 512, 512)).astype(np.float32) * 0.02,
        U_h=rng.normal(size=(12, 512, 512)).astype(np.float32) * 0.04,
        b_zr=np.zeros((12, 1024), np.float32),
        b_h=np.zeros((12, 512), np.float32),
        Wout=rng.normal(size=(12, 512, 512)).astype(np.float32) * 0.02,
        norm_gamma=np.ones((512,), np.float32),
        norm_beta=np.zeros((512,), np.float32),
    )
    out = kernel(**ins)
    print(out.shape, out.dtype, np.abs(out).max())



# revision 18
# speedup vs baseline: 1.9879x; 1.0171x over previous
"""Trainium2 Bass kernel for nn_CudaGRULM — lag-1 interleaved layer pipeline.

Layer pipeline over 6 cores (core k hosts layers 2k, 2k+1), but cell B runs
ONE ROUND BEHIND cell A: at round r core k scans chunk r-2k through layer 2k
(cell A) and chunk r-2k-1 through layer 2k+1 (cell B, consuming the Aout
written last round). The two scans are independent, so their per-step
DVE/ACT chains hide under each other's matmuls (the serial version
alternated MM-group -> chain -> MM-group and left the PE idle most of the
chain time). Chunk c passes layer 2k at round c+2k, layer 2k+1 at c+2k+1;
pipeline depth 11, R = NC + 11 rounds.

Scan step structure (per cell): r/z/h psum groups live in SEPARATE banks
(bank-aware collision tracking would otherwise serialize each step around
one bank); the x-projections are prefolded into psum by identity matmuls so
sigmoid/tanh read psum directly; 1-z = sigmoid(-x) lets a = (1-z)*prev be
precomputed off the critical path; h_t = a + b lands as a bf16 ring write
(DVE) plus an fp32 carried state.

Everything is python-unrolled with static access patterns: operands with
register-offset APs cost ~195ns per matmul vs ~40-60ns static.

S=32 so a chunk is one 512-column group: input projections for cell B of
round r+1 are emitted in round r's tail, and the A-side head (AG-gated DMA +
LN + inproj) hides under cell B's first scan steps.
"""

from contextlib import ExitStack

import numpy as np

import concourse.bass as bass
import concourse.bacc as bacc
import concourse.tile as tile
from concourse import mybir
from concourse.bass_utils import run_bass_kernel_spmd

FP = mybir.dt.float32
BF = mybir.dt.bfloat16
AF = mybir.ActivationFunctionType
ALU = mybir.AluOpType

# test/experiment knobs (kernel() itself always runs the default config):
# NO_COLLECTIVES replaces the AllGather with a local DRAM copy (wrong output,
# used for timing A/B and TimelineSim); SCAN_MM_ONLY keeps only the scan's
# matmuls (timing diagnostic, garbage output).
NO_COLLECTIVES = False
SCAN_MM_ONLY = False


class Cfg:
    def __init__(self, V=256, D=512, DEPTH=12, DI=512, B=16, T=2048,
                 n_cores=8, S=32, EPS=1e-5):
        self.V, self.D, self.DEPTH, self.DI, self.B, self.T = V, D, DEPTH, DI, B, T
        self.n_cores = n_cores
        self.S = S                      # scan steps per chunk
        self.NC = T // S                # chunks
        self.CC = S * B                 # chunk cols (token-major: col = t*B+b)
        self.EPS = EPS
        self.KD = D // 128              # 4
        self.KV = V // 128              # 2
        self.MZR = 2 * DI // 128        # 8
        self.MH = DI // 128             # 4
        # chunk c passes layer l at round c+l with 2 layers/core and lag-1:
        # A_k at round c+2k, B_k at c+2k+1; logits (core 5 B output) at
        # round c+11 -> last round = (NC-1)+11
        self.R = self.NC + 11
        assert D == DI and self.CC == 512 and T % S == 0


def build_kernel(ctx: ExitStack, tc: "tile.TileContext", outs, ins, obs, cfg: Cfg):
    nc = tc.nc
    c = cfg
    KD, KV, MZR, MH, B, CC, S = c.KD, c.KV, c.MZR, c.MH, c.B, c.CC, c.S
    MR = MZR // 2
    lg_out = outs["logits"]

    persist = ctx.enter_context(tc.tile_pool(name="persist", bufs=1))
    sb = ctx.enter_context(tc.tile_pool(name="sb", bufs=2))
    dram = ctx.enter_context(tc.tile_pool(name="dram", bufs=2, space="DRAM"))
    # scan psum: one bank per (cell, group) so PE writes never bank-collide
    # with ACT/DVE reads of another group; bufs=1 is safe because each
    # group's reader finishes well before the next step's first write
    ps_sc = ctx.enter_context(tc.tile_pool(name="ps_sc", bufs=1, space="PSUM"))
    ps_pr = ctx.enter_context(tc.tile_pool(name="ps_pr", bufs=2, space="PSUM"))

    # ---- constants ----
    iota2 = persist.tile([128, KV], FP)
    nc.sync.dma_start(iota2[:], ins["iota2"][:])
    onec_f = persist.tile([1, 128], FP)
    nc.sync.dma_start(onec_f[:], ins["ones_col"][:])
    onec_b = persist.tile([1, 128], BF)
    nc.vector.tensor_copy(onec_b[:], onec_f[:])
    onek_f = persist.tile([128, 1], FP)
    nc.sync.dma_start(onek_f[:], ins["ones_k"][:])
    onek_b = persist.tile([128, 1], BF)
    nc.vector.tensor_copy(onek_b[:], onek_f[:])
    e_sb = persist.tile([128, KV, c.D], BF)
    nc.sync.dma_start(e_sb[:], ins["E_lhsT"][:])
    et_sb = persist.tile([128, KD, c.V], BF)
    nc.sync.dma_start(et_sb[:], ins["ET_rhs"][:])
    bv_sb = persist.tile([1, c.V], BF)
    nc.sync.dma_start(bv_sb[:], ins["bv_row"][:])
    eps_sb = persist.tile([1, 1], FP)
    nc.vector.memset(eps_sb[:], float(c.EPS))
    m_emb = persist.tile([128, 1], FP)
    nc.sync.dma_start(m_emb[:], ins["m_emb"][:])
    m_keep = persist.tile([128, 1], FP)
    nc.sync.dma_start(m_keep[:], ins["m_keep"][:])
    m_stA = persist.tile([128, c.R], FP)
    nc.sync.dma_start(m_stA[:], ins["m_stA"][:])
    m_stB = persist.tile([128, c.R], FP)
    nc.sync.dma_start(m_stB[:], ins["m_stB"][:])
    ident = persist.tile([128, 128], BF)
    nc.sync.dma_start(ident[:], ins["ident"][:])

    # ---- per-slot weights (resident the whole run) ----
    def load_w(pfx):
        shapes = dict(uzr=([128, KD, 2 * c.DI], BF), uh=([128, KD, c.DI], BF),
                      wzr=([128, KD, 2 * c.DI], BF), wh=([128, KD, c.DI], BF),
                      wo=([128, KD, c.D], BF), bzr=([128, MZR], FP),
                      bh=([128, MH], FP))
        srcs = dict(uzr="uzrT", uh="uhT", wzr="wzrT", wh="whT", wo="woT",
                    bzr="bzr", bh="bh")
        w = {}
        for kk, (shp_, dt_) in shapes.items():
            w[kk] = persist.tile(shp_, dt_, name=pfx + kk)
            nc.sync.dma_start(w[kk][:], ins[pfx + srcs[kk]][:])
        return w

    WA = load_w("LA_")
    WB = load_w("LB_")

    # ---- streaming buffers ----
    inA = persist.tile([128, KD, CC], BF)        # cell A input chunk
    AoutD = [persist.tile([128, KD, CC], BF, name=f"Aout{i}")
             for i in range(2)]                  # A output, by round parity
    Bout = persist.tile([128, KD, CC], BF)       # cell B output (AG payload)
    stA = persist.tile([128, KD, B], FP)         # carried scan states
    stB = persist.tile([128, KD, B], FP)
    hsA = persist.tile([128, KD, (S + 1) * B], BF)   # scan out rings
    hsB = persist.tile([128, KD, (S + 1) * B], BF)
    xzrA = persist.tile([128, MZR, CC], BF)          # input projections
    xhA = persist.tile([128, MH, CC], BF)
    xzrB = persist.tile([128, MZR, CC], BF)
    xhB = persist.tile([128, MH, CC], BF)
    nc.vector.memset(inA[:], 0.0)
    nc.vector.memset(AoutD[0][:], 0.0)
    nc.vector.memset(AoutD[1][:], 0.0)
    nc.vector.memset(Bout[:], 0.0)
    nc.vector.memset(stA[:], 0.0)
    nc.vector.memset(stB[:], 0.0)

    # per-core row offset of the previous rank inside AG outputs
    if NO_COLLECTIVES:
        off = c.n_cores - 1
    else:
        off_reg = nc.sync.alloc_register("prev_off")
        nc.sync.reg_load(off_reg, ins["prev_off"][0:1, 0:1])
        off = nc.sync.snap(off_reg, donate=True, min_val=0, max_val=c.n_cores - 1)

    def layer_norm(tsrc, hn, sqtag):
        """LN over partitions (D) of the full CC cols of tsrc into hn."""
        mean_ps = ps_pr.tile([128, 512], FP, tag="pr")
        for k in range(KD):
            nc.tensor.matmul(mean_ps[0:1, :], onek_b[:], tsrc[:, k, :],
                             start=(k == 0), stop=(k == KD - 1))
        sq = sb.tile([128, KD, 512], BF, tag=sqtag, bufs=2)
        for k in range(KD):
            nc.scalar.activation(sq[:, k, :], tsrc[:, k, :], AF.Square)
        mean_r = sb.tile([1, 512], FP, tag="lnrow", bufs=8)
        nc.vector.tensor_scalar(mean_r[:], mean_ps[0:1, :], 1.0 / c.D, None,
                                ALU.mult)
        sq_ps = ps_pr.tile([128, 512], FP, tag="pr")
        for k in range(KD):
            nc.tensor.matmul(sq_ps[0:1, :], onek_b[:], sq[:, k, :],
                             start=(k == 0), stop=(k == KD - 1))
        msq_r = sb.tile([1, 512], FP, tag="lnrow", bufs=8)
        nc.vector.tensor_scalar(msq_r[:], sq_ps[0:1, :], 1.0 / c.D, None,
                                ALU.mult)
        var_r = sb.tile([1, 512], FP, tag="lnrow", bufs=8)
        nc.vector.tensor_tensor(var_r[:], mean_r[:], mean_r[:], ALU.mult)
        nc.vector.tensor_tensor(var_r[:], msq_r[:], var_r[:], ALU.subtract)
        std_r = sb.tile([1, 512], FP, tag="lnrow", bufs=8)
        nc.scalar.activation(std_r[:], var_r[:], AF.Sqrt, bias=eps_sb[:])
        rstd_r = sb.tile([1, 512], FP, tag="lnrow", bufs=8)
        nc.vector.reciprocal(rstd_r[:], std_r[:])
        mr_r = sb.tile([1, 512], FP, tag="lnrow", bufs=8)
        nc.vector.tensor_tensor(mr_r[:], mean_r[:], rstd_r[:], ALU.mult)
        rb_ps = ps_pr.tile([128, 512], FP, tag="pr")
        nc.tensor.matmul(rb_ps[:], onec_f[:], rstd_r[:], start=True, stop=True)
        mrb_ps = ps_pr.tile([128, 512], FP, tag="pr")
        nc.tensor.matmul(mrb_ps[:], onec_f[:], mr_r[:], start=True, stop=True)
        for k in range(KD):
            nc.vector.tensor_tensor(hn[:, k, :], tsrc[:, k, :], rb_ps[:],
                                    ALU.mult)
            nc.vector.tensor_tensor(hn[:, k, :], hn[:, k, :], mrb_ps[:],
                                    ALU.subtract)

    def inproj(tsrc, W, xzr_t, xh_t, sqtag):
        """LN(tsrc) then Wzr/Wh projections into xzr_t/xh_t (full chunk)."""
        hn = sb.tile([128, KD, 512], BF, tag="hn" + sqtag, bufs=2)
        layer_norm(tsrc, hn, sqtag)
        for m in range(MZR):
            px = ps_pr.tile([128, 512], FP, tag="pr")
            for k in range(KD):
                nc.tensor.matmul(px[:], W["wzr"][:, k, m * 128:(m + 1) * 128],
                                 hn[:, k, :], start=(k == 0),
                                 stop=(k == KD - 1))
            nc.scalar.activation(xzr_t[:, m, :], px[:], AF.Identity,
                                 bias=W["bzr"][:, m:m + 1])
        for m in range(MH):
            px = ps_pr.tile([128, 512], FP, tag="pr")
            for k in range(KD):
                nc.tensor.matmul(px[:], W["wh"][:, k, m * 128:(m + 1) * 128],
                                 hn[:, k, :], start=(k == 0),
                                 stop=(k == KD - 1))
            nc.scalar.activation(xh_t[:, m, :], px[:], AF.Identity,
                                 bias=W["bh"][:, m:m + 1])

    def cell_start(cs, r):
        """Mask the carried state for fill/drain and seed ring col 0."""
        nc.vector.tensor_scalar(cs["st"][:], cs["st"][:],
                                cs["m_st"][:, r:r + 1], None, ALU.mult)
        nc.vector.tensor_copy(cs["hs"][:, :, 0:B], cs["st"][:])
        cs["prev"] = cs["st"]

    def scan_mms(cs, t):
        """Identity prefolds + r/z U-matmuls of step t (one cell)."""
        cin = slice(t * B, (t + 1) * B)
        W, hs, xzr_t, xh_t, tg = (cs["W"], cs["hs"], cs["xzr"], cs["xh"],
                                  cs["tag"])
        rps = ps_sc.tile([128, MH * B], FP, tag="r" + tg)
        zps = ps_sc.tile([128, MR * B], FP, tag="z" + tg)
        hps = ps_sc.tile([128, MH * B], FP, tag="h" + tg)
        cs["rps"], cs["zps"], cs["hps"] = rps, zps, hps
        # prefold x-projections into psum via identity matmuls: the first
        # clears the bank (start=True); the rest overwrite their own
        # has_written-clear slots. Sigmoid/tanh then read psum directly.
        for m in range(MH):
            nc.tensor.matmul(rps[:, m * B:(m + 1) * B], ident[:],
                             xzr_t[:, MR + m, cin], start=(m == 0),
                             stop=False, skip_group_check=True)
        for m in range(MH):
            nc.tensor.matmul(hps[:, m * B:(m + 1) * B], ident[:],
                             xh_t[:, m, cin], start=(m == 0),
                             stop=False, skip_group_check=True)
        for m in range(MR, MZR):
            for k in range(KD):
                nc.tensor.matmul(rps[:, (m - MR) * B:(m - MR + 1) * B],
                                 W["uzr"][:, k, m * 128:(m + 1) * 128],
                                 hs[:, k, cin], start=False,
                                 stop=(m == MZR - 1 and k == KD - 1),
                                 skip_group_check=True)
        for m in range(MR):
            for k in range(KD):
                nc.tensor.matmul(zps[:, m * B:(m + 1) * B],
                                 W["uzr"][:, k, m * 128:(m + 1) * 128],
                                 hs[:, k, cin],
                                 start=(k == 0), stop=(k == KD - 1))

    def scan_rchain(cs, t):
        """r sigmoid + rh; z gates + a = (1-z)*prev (off critical path)."""
        cin = slice(t * B, (t + 1) * B)
        tg = cs["tag"]
        if SCAN_MM_ONLY:
            return
        za_r = sb.tile([128, MH, B], FP, tag="zar" + tg, bufs=3)
        nc.scalar.activation(za_r[:], cs["rps"][:], AF.Sigmoid)
        rh = sb.tile([128, KD, B], BF, tag="rh" + tg, bufs=3)
        nc.vector.tensor_tensor(rh[:], za_r[:], cs["prev"][:], ALU.mult)
        cs["rh"] = rh
        zs_z = sb.tile([128, MR, B], FP, tag="zsz" + tg, bufs=3)
        nc.vector.tensor_tensor(zs_z[:], cs["zps"][:], cs["xzr"][:, 0:MR, cin],
                                ALU.add)
        za_z = sb.tile([128, MR, B], FP, tag="zaz" + tg, bufs=3)
        nc.scalar.activation(za_z[:], zs_z[:], AF.Sigmoid)
        zc = sb.tile([128, MR, B], FP, tag="zc" + tg, bufs=3)
        nc.scalar.activation(zc[:], zs_z[:], AF.Sigmoid, scale=-1.0)
        a = sb.tile([128, KD, B], FP, tag="a" + tg, bufs=3)
        nc.vector.tensor_tensor(a[:], zc[:], cs["prev"][:], ALU.mult)
        cs["za_z"], cs["a"] = za_z, a

    def scan_tail(cs, t):
        """Candidate matmuls + tanh + h_t = a + z*hc; ring write + state."""
        cout = slice((t + 1) * B, (t + 2) * B)
        W, tg = cs["W"], cs["tag"]
        rhs = cs["hs"][:, :, slice(t * B, (t + 1) * B)] if SCAN_MM_ONLY \
            else cs["rh"]
        for m in range(MH):
            for k in range(KD):
                nc.tensor.matmul(cs["hps"][:, m * B:(m + 1) * B],
                                 W["uh"][:, k, m * 128:(m + 1) * 128],
                                 rhs[:, k, :] if SCAN_MM_ONLY
                                 else rhs[:, k, :],
                                 start=False,
                                 stop=(m == MH - 1 and k == KD - 1),
                                 skip_group_check=True)
        if SCAN_MM_ONLY:
            return
        hc = sb.tile([128, MH, B], FP, tag="hc" + tg, bufs=3)
        nc.scalar.activation(hc[:], cs["hps"][:], AF.Tanh)
        b = sb.tile([128, KD, B], FP, tag="b" + tg, bufs=3)
        nc.vector.tensor_tensor(b[:], cs["za_z"][:], hc[:], ALU.mult)
        # bf16 ring write on DVE (critical path for the next step's MMs);
        # fp32 carried state as a second DVE op
        nc.vector.tensor_tensor(cs["hs"][:, :, cout], cs["a"][:], b[:],
                                ALU.add)
        nxt = sb.tile([128, KD, B], FP, tag="nx" + tg, bufs=3)
        nc.vector.tensor_tensor(nxt[:], cs["a"][:], b[:], ALU.add)
        cs["prev"] = nxt

    def outproj_half(cs, dst, src_res, half):
        """Wout @ hs + residual for one 256-col half of the chunk."""
        W, hs = cs["W"], cs["hs"]
        cols = slice(half * 256, (half + 1) * 256)
        for dm in range(KD):
            po = ps_pr.tile([128, 512], FP, tag="pr")
            for k in range(KD):
                nc.tensor.matmul(po[:, 0:256],
                                 W["wo"][:, k, dm * 128:(dm + 1) * 128],
                                 hs[:, k, B + half * 256:B + (half + 1) * 256],
                                 start=(k == 0), stop=(k == KD - 1))
            nc.vector.tensor_tensor(dst[:, dm, cols], src_res[:, dm, cols],
                                    po[:, 0:256], ALU.add)

    def embed_blend(r):
        """One-hot embedding of chunk r blended into inA (core 0 only)."""
        xcol = min(r, c.NC - 1) * CC
        x_row = sb.tile([1, CC], FP, tag="xrow", bufs=2)
        nc.sync.dma_start(x_row[:], ins["x_tb"][:, xcol:xcol + CC])
        xb_ps = ps_pr.tile([128, 512], FP, tag="pr")
        nc.tensor.matmul(xb_ps[:], onec_f[:], x_row[:], start=True, stop=True)
        ohs = []
        for vc in range(KV):
            oh = sb.tile([128, 512], BF, tag=f"oh{vc}", bufs=2)
            nc.vector.tensor_scalar(oh[:], xb_ps[:], iota2[:, vc:vc + 1],
                                    m_emb[:, 0:1], ALU.is_equal, ALU.mult)
            ohs.append(oh)
        for dm in range(KD):
            px = ps_pr.tile([128, 512], FP, tag="pr")
            for vc in range(KV):
                nc.tensor.matmul(px[:], e_sb[:, vc, dm * 128:(dm + 1) * 128],
                                 ohs[vc][:], start=(vc == 0),
                                 stop=(vc == KV - 1))
            nc.vector.tensor_scalar(inA[:, dm, :], inA[:, dm, :],
                                    m_keep[:, 0:1], None, ALU.mult)
            nc.vector.tensor_tensor(inA[:, dm, :], inA[:, dm, :], px[:],
                                    ALU.add)

    def logits_block(r):
        """Final LN + tied lm_head over Bout; rows r*CC of lg_out."""
        hnF = sb.tile([128, KD, 512], BF, tag="hnF", bufs=2)
        layer_norm(Bout, hnF, "sqF")
        for t4 in range(4):
            pl = ps_pr.tile([128, 512], FP, tag="pr")
            for k in range(KD):
                nc.tensor.matmul(pl[:, 0:c.V],
                                 hnF[:, k, t4 * 128:(t4 + 1) * 128],
                                 et_sb[:, k, :], start=(k == 0), stop=False)
            nc.tensor.matmul(pl[:, 0:c.V], onec_b[:], bv_sb[:], start=False,
                             stop=True)
            out_sb = sb.tile([128, c.V], FP, tag="osb", bufs=3)
            nc.vector.tensor_copy(out_sb[:], pl[:, 0:c.V])
            r0 = r * CC + t4 * 128
            nc.sync.dma_start(lg_out[r0:r0 + 128, :], out_sb[:])

    # cell states
    csA = dict(W=WA, hs=hsA, st=stA, xzr=xzrA, xh=xhA, m_st=m_stA, tag="A")
    csB = dict(W=WB, hs=hsB, st=stB, xzr=xzrB, xh=xhB, m_st=m_stB, tag="B")

    # ================= rounds =================
    # prologue: cell B's round-0 projections (input is the zeroed Aout[1])
    inproj(AoutD[1], WB, xzrB, xhB, "sqB")

    for r in range(c.R):
        AoutW = AoutD[r % 2]          # A writes this round
        AoutR = AoutD[(r + 1) % 2]    # B reads (written last round)
        # ---- A-side head (hides under B's first scan steps) ----
        if r >= 1:
            nc.sync.dma_start(inA[:],
                              obs[r - 1][bass.ds(off * 128, 128), :, :])
        if r < c.NC:
            embed_blend(r)
        cell_start(csB, r)
        cell_start(csA, r)
        # emit B's first steps before A's projections so the PE has scan
        # work while the A head (DMA + LN + proj) resolves
        lead = 6
        for t in range(lead):
            scan_mms(csB, t)
            scan_rchain(csB, t)
            scan_tail(csB, t)
        inproj(inA, WA, xzrA, xhA, "sqA")
        # ---- interleaved supersteps ----
        for t in range(S):
            ta = t
            if t + lead < S:
                scan_mms(csB, t + lead)
                scan_rchain(csB, t + lead)
            scan_mms(csA, ta)
            scan_rchain(csA, ta)
            if t + lead < S:
                scan_tail(csB, t + lead)
            scan_tail(csA, ta)
            if t == 17 and not SCAN_MM_ONLY:
                # first halves of both output projections (cols 0..255 need
                # scan steps 0..15 only)
                outproj_half(csB, Bout, AoutR, 0)
                outproj_half(csA, AoutW, inA, 0)
        # ---- tail ----
        nc.vector.tensor_copy(stB[:], csB["prev"][:])
        nc.vector.tensor_copy(stA[:], csA["prev"][:])
        if not SCAN_MM_ONLY:
            outproj_half(csB, Bout, AoutR, 1)
            outproj_half(csA, AoutW, inA, 1)
        else:
            for half in (0, 1):
                outproj_half(csB, Bout, AoutR, half)
                outproj_half(csA, AoutW, inA, half)
        # AG payload + collective, then next-round prep on the PE
        if r < c.R - 1:
            ib = dram.tile([128, KD, CC], BF, tag="ib")
            nc.sync.dma_start(ib[:], Bout[:])
            if not NO_COLLECTIVES:
                nc.gpsimd.collective_compute(
                    "AllGather", ALU.bypass,
                    replica_groups=[list(range(c.n_cores))],
                    ins=[ib.opt()], outs=[obs[r].opt()],
                )
            else:
                nc.sync.dma_start(obs[r][0:128, :, :], ib[:])
        if r >= 11:
            logits_block(r)
        if r < c.R - 1:
            inproj(AoutW, WB, xzrB, xhB, "sqB")


# ======================= host side =======================

def _pack_lhsT(m, kchunks, dtype):
    K, J = m.shape
    assert K == kchunks * 128
    return np.ascontiguousarray(m.reshape(kchunks, 128, J).transpose(1, 0, 2),
                                dtype=dtype)


def prep_inputs(inputs, cfg: "Cfg"):
    import ml_dtypes
    bf = ml_dtypes.bfloat16
    c = cfg
    f8 = np.float64
    x = np.asarray(inputs["x"])
    emb = np.asarray(inputs["embedding"], f8)
    ln_g = np.asarray(inputs["ln_gamma"], f8)
    ln_b = np.asarray(inputs["ln_beta"], f8)
    Win = np.asarray(inputs["Win"], f8)
    W_zr = np.asarray(inputs["W_zr"], f8)
    U_zr = np.asarray(inputs["U_zr"], f8)
    W_h = np.asarray(inputs["W_h"], f8)
    U_h = np.asarray(inputs["U_h"], f8)
    b_zr = np.asarray(inputs["b_zr"], f8)
    b_h = np.asarray(inputs["b_h"], f8)
    Wout = np.asarray(inputs["Wout"], f8)
    ng = np.asarray(inputs["norm_gamma"], f8)
    nb = np.asarray(inputs["norm_beta"], f8)

    def layer_pack(l):
        if l >= c.DEPTH:
            z2 = np.zeros((128, c.KD, 2 * c.DI), bf)
            z1 = np.zeros((128, c.KD, c.DI), bf)
            return dict(uzrT=z2, uhT=z1, wzrT=z2.copy(), whT=z1.copy(),
                        woT=np.zeros((128, c.KD, c.D), bf),
                        bzr=np.zeros((128, c.MZR), np.float32),
                        bh=np.zeros((128, c.MH), np.float32))
        Wzr_eff = W_zr[l] @ Win[l]
        bzr_eff = Wzr_eff @ ln_b[l] + b_zr[l]
        Wzr_eff = Wzr_eff * ln_g[l][None, :]
        Wh_eff = W_h[l] @ Win[l]
        bh_eff = Wh_eff @ ln_b[l] + b_h[l]
        Wh_eff = Wh_eff * ln_g[l][None, :]
        return dict(
            uzrT=_pack_lhsT(U_zr[l].T, c.KD, bf),
            uhT=_pack_lhsT(U_h[l].T, c.KD, bf),
            wzrT=_pack_lhsT(Wzr_eff.T, c.KD, bf),
            whT=_pack_lhsT(Wh_eff.T, c.KD, bf),
            woT=_pack_lhsT(Wout[l].T, c.KD, bf),
            bzr=np.ascontiguousarray(bzr_eff.reshape(c.MZR, 128).T, np.float32),
            bh=np.ascontiguousarray(bh_eff.reshape(c.MH, 128).T, np.float32),
        )

    shared = {}
    shared["E_lhsT"] = np.ascontiguousarray(
        emb.reshape(c.KV, 128, c.D).transpose(1, 0, 2), dtype=bf)
    shared["ET_rhs"] = _pack_lhsT((emb * ng[None, :]).T, c.KD, bf)
    shared["bv_row"] = np.ascontiguousarray((emb @ nb)[None, :], dtype=bf)
    shared["iota2"] = np.ascontiguousarray(
        (np.arange(128)[:, None] + 128 * np.arange(c.KV)[None, :]), np.float32)
    shared["ones_col"] = np.ones((1, 128), np.float32)
    shared["ones_k"] = np.ones((128, 1), np.float32)
    shared["ident"] = np.eye(128, dtype=bf)
    shared["x_tb"] = np.ascontiguousarray(x.T.reshape(1, -1), dtype=np.float32)

    in_maps = []
    for core in range(c.n_cores):
        m = dict(shared)
        la = 2 * core if core < 6 else c.DEPTH       # >= DEPTH -> zero weights
        lb = 2 * core + 1 if core < 6 else c.DEPTH
        for kk, v in layer_pack(la).items():
            m["LA_" + kk] = v
        for kk, v in layer_pack(lb).items():
            m["LB_" + kk] = v
        # A_k holds chunk r-2k at round r (reset at r=2k); B_k holds chunk
        # r-2k-1 (reset at r=2k+1)
        mstA = np.zeros((128, c.R), np.float32)
        mstB = np.zeros((128, c.R), np.float32)
        if core < 6:
            for r in range(c.R):
                if 2 * core < r < 2 * core + c.NC:
                    mstA[:, r] = 1.0
                if 2 * core + 1 < r < 2 * core + 1 + c.NC:
                    mstB[:, r] = 1.0
        m["m_stA"] = mstA
        m["m_stB"] = mstB
        m["m_emb"] = np.full((128, 1), 1.0 if core == 0 else 0.0, np.float32)
        m["m_keep"] = np.full((128, 1), 0.0 if core == 0 else 1.0, np.float32)
        m["prev_off"] = np.array([[(core - 1) % c.n_cores]], np.uint32)
        in_maps.append(m)
    return in_maps, shared


def declare_tensors(nc, cfg: "Cfg", in_map0):
    c = cfg
    ins = {}
    for name, arr in in_map0.items():
        dt = mybir.dt.from_np(arr.dtype)
        ins[name] = nc.dram_tensor(name, list(arr.shape), dt,
                                   kind="ExternalInput").ap()
    outs = {}
    outs["logits"] = nc.dram_tensor("logits", [c.R * c.CC, c.V], FP,
                                    kind="ExternalOutput").ap()
    return outs, ins


_CACHE = {}


def build_program(cfg: "Cfg", in_map0):
    key = (cfg.DEPTH, cfg.T, cfg.S, cfg.n_cores, NO_COLLECTIVES, SCAN_MM_ONLY)
    if key in _CACHE:
        return _CACHE[key]
    nc = bacc.Bacc("TRN2", target_bir_lowering=False, debug=False,
                   num_devices=cfg.n_cores)
    outs, ins = declare_tensors(nc, cfg, in_map0)
    with tile.TileContext(nc) as tc:
        with ExitStack() as ctx:
            shp = ctx.enter_context(tc.tile_pool(name="shdram", bufs=1,
                                                 space="DRAM"))
            obs = [shp.tile([cfg.n_cores * 128, cfg.KD, cfg.CC], BF,
                            addr_space="Shared", name=f"ob{r}")
                   for r in range(cfg.R)]
            build_kernel(ctx, tc, outs, ins, obs, cfg)
    nc.compile()
    _CACHE[key] = nc
    return nc


def kernel(**inputs) -> np.ndarray:
    cfg = Cfg()
    in_maps, shared = prep_inputs(inputs, cfg)
    nc = build_program(cfg, in_maps[0])
    res = run_bass_kernel_spmd(nc, in_maps, core_ids=list(range(cfg.n_cores)))
    lg = res.results[5]["logits"]          # [R*CC, V]
    c = cfg
    out = np.empty((c.B, c.T, c.V), np.float32)
    for ch in range(c.NC):
        blk = lg[(11 + ch) * c.CC:(12 + ch) * c.CC, :]
        blk = blk.reshape(c.S, c.B, c.V).transpose(1, 0, 2)
        out[:, ch * c.S:(ch + 1) * c.S, :] = blk
    return np.ascontiguousarray(out)


if __name__ == "__main__":
    rng = np.random.default_rng(0)
    ins = dict(
        x=rng.integers(0, 256, size=(16, 2048)),
        embedding=rng.normal(size=(256, 512)).astype(np.float32) * 0.02,
        ln_gamma=np.ones((12, 512), np.float32),
        ln_beta=np.zeros((12, 512), np.float32),
        Win=rng.normal(size=(12, 512, 512)).astype(np.float32) * 0.02,
        W_zr=rng.normal(size=(12, 1024, 512)).astype(np.float32) * 0.02,
        U_zr=rng.normal(size=(12, 1024, 512)).astype(np.float32) * 0.04,
        W_h=rng.normal(size=(12, 512, 512)).astype(np.float32) * 0.02,
        U_h=rng.normal(size=(12, 512, 512)).astype(np.float32) * 0.04,
        b_zr=np.zeros((12, 1024), np.float32),
        b_h=np.zeros((12, 512), np.float32),
        Wout=rng.normal(size=(12, 512, 512)).astype(np.float32) * 0.02,
        norm_gamma=np.ones((512,), np.float32),
        norm_beta=np.zeros((512,), np.float32),
    )
    out = kernel(**ins)
    print(out.shape, out.dtype, np.abs(out).max())
